# revision 39
# baseline (speedup 1.0000x reference)
"""Trainium2 Bass kernel for a 4-head GAT layer (N=4096, D=256, O=64, H=4).

Math (reference):
    feat[h] = X @ W[h]                                  [N, O]
    s[h,i] = feat[h,i] @ a_src[h],  t[h,j] = feat[h,j] @ a_dst[h]
    score[h,i,j] = leaky_relu(s_i + t_j, 0.2), masked by A>0, softmax over j
    out[i, h*O+o] = sum_j attn[h,i,j] feat[h,j,o] + b[h,o]

Key identity used on-device (max-trick):
    exp(leaky_relu(x)) = max(e^x, e^{0.2x});  with x = s_i + t_j and the
    row-constant e^{0.2 s_i} divided out (it cancels in the softmax ratio):
      G[i,j] = A_ij * max(e^{0.8 s_i} * e^{t_j}, e^{0.2 t_j})
      out[i] = (G @ [feat|1])[0:64] / (G @ [feat|1])[64]
    Per (head, j-tile) G is built with exactly two DVE ops:
      u = tensor_scalar(wbc_h, v_j, q_j, mult, max)   (4x mode, 194ns)
      G = u * A_row                                   (2x mode, batched 4 heads)
    where wbc_h = e^{0.8 s_i} broadcast along partitions (built by a matmul
    with a repeated -w_src stationary, then one ACT exp pass), v = e^t,
    q = e^{0.2t} are per-partition scalars.

    The masked matmul runs moving=G (512-wide stream), stationary=[feat|1]
    (65 cols), accumulating [65, 512] per head in one PSUM bank over all 32
    j-tiles; the result is PE-transposed back to [i, 65] for the epilogue
    (reciprocal of the ones-column times the feature columns).

Sharding: destination rows are split 512/core across 8 cores; source-side
features (all N) are recomputed per core (cheap).  No collectives.
b is always zero in setup_inputs but is added on the host anyway.
"""

from contextlib import ExitStack

import numpy as np

import concourse.bass as bass
import concourse.tile as tile
import concourse.mybir as mybir
from concourse import bacc
from concourse.bass_utils import run_bass_kernel_spmd
from concourse.masks import make_identity

P = 128
IN_DIM = 256
OUT_DIM = 64
HEADS = 4
N_TOTAL = 4096
N_CORES = 8
ROWS = N_TOTAL // N_CORES  # 512

F32 = mybir.dt.float32
F16 = mybir.dt.float16

AL = mybir.AluOpType
AF = mybir.ActivationFunctionType

GRP = 66          # [feat(64) | 1 | 1] per head in the fe panel (even => 4B aligned)
FET_C = 4 * GRP   # 264 cols per n-tile in fet


def build_program(n_total=N_TOTAL, rows=ROWS, num_devices=N_CORES):
    """Build the per-core SPMD program (same program on all cores; per-core
    data arrives via the input map)."""
    ntiles = n_total // P   # source-node tiles (j)
    nib = rows // P         # destination row blocks per core
    njt = ntiles

    nc = bacc.Bacc("TRN2", target_bir_lowering=False, debug=False,
                   num_devices=num_devices)

    # SMALL = [xtown(512) | w8(260) | wsrcb(512)] packed -> 2 DMA descriptors
    SM_C = rows + 260 + 4 * P
    XT = nc.dram_tensor("XT", [IN_DIM, n_total], F16, kind="ExternalInput")
    SMALL = nc.dram_tensor("SMALL", [IN_DIM, SM_C], F16, kind="ExternalInput")
    AT = nc.dram_tensor("AT", [n_total, rows], F16, kind="ExternalInput")
    OUT = nc.dram_tensor("OUT", [rows, HEADS * OUT_DIM], F32,
                         kind="ExternalOutput")

    with tile.TileContext(nc) as tc, ExitStack() as ctx:
        big = ctx.enter_context(tc.tile_pool(name="big", bufs=1))

        # ---- Phase 0: load everything ----
        # Small tensors first: the sbc / feat matmuls need them, and DMA
        # queues drain FIFO -- putting the 6MB of XT/AT ahead of them stalls
        # all compute behind ~20us of bulk DMA.
        sm_sb = big.tile([P, 2 * SM_C], F16, tag="sm")
        for d in range(2):
            nc.sync.dma_start(sm_sb[:, d * SM_C:(d + 1) * SM_C],
                              SMALL[d * P:(d + 1) * P, :])

        def xtown_sl(d, c0, c1):
            return sm_sb[:, d * SM_C + c0: d * SM_C + c1]

        def w8_sl(d, c0, c1):
            return sm_sb[:, d * SM_C + rows + c0: d * SM_C + rows + c1]

        def wsrcb_sl(d, c0, c1):
            base = d * SM_C + rows + 260
            return sm_sb[:, base + c0: base + c1]
        # Interleave xt chunks with the first at tiles so the jt-loop can
        # start while the bulk of A is still in flight.
        xt_sb = big.tile([P, 2 * n_total], F16, tag="xt")
        at_sb = big.tile([P, njt * rows], F16, tag="at")

        def load_xt(c, nch=8):
            w = n_total // nch
            for d in range(2):
                nc.sync.dma_start(
                    xt_sb[:, d * n_total + c * w: d * n_total + (c + 1) * w],
                    XT[d * P:(d + 1) * P, c * w:(c + 1) * w])

        def load_at(jt):
            nc.sync.dma_start(at_sb[:, jt * rows:(jt + 1) * rows],
                              AT[jt * P:(jt + 1) * P, :])

        load_xt(0)
        load_xt(1)
        load_at(0)
        load_at(1)
        for c in range(2, 8):
            load_xt(c)
            load_at(c)
        for jt in range(8, njt):
            load_at(jt)

        # identity for the PE transposes (built on idle gpsimd)
        idn = big.tile([P, P], F32, tag="idn")
        make_identity(nc, idn[:])

        # ---- Phase 1a: sbc = -s_i broadcast along partitions, per head ----
        # sbc[:, h*rows + i] = -s_src[h, i]  (every partition)
        sbc = big.tile([P, 4 * rows], F16, tag="sbc")
        wbc = big.tile([P, 4 * rows], F16, tag="wbc")
        # one matmul per (head, d): stationary = repeated -w_src block,
        # moving = all 512 own-row columns; 8 big matmuls instead of 32 tiny
        # ones (the cold PE runs at 0.65GHz — stream time dominates)
        with tc.tile_pool(name="psb", bufs=2, space=bass.MemorySpace.PSUM) as psb:
            for h in range(HEADS):
                ps = psb.tile([P, rows], F32, tag="ps_sb")
                for d in range(2):
                    nc.tensor.matmul(
                        ps[:],
                        wsrcb_sl(d, h * P, (h + 1) * P),
                        xtown_sl(d, 0, rows),
                        start=(d == 0), stop=(d == 1))
                nc.vector.tensor_copy(
                    sbc[:, h * rows:(h + 1) * rows], ps[:])
                # wbc = e^{0.8 s} = exp(-0.8 * sbc), per head so the jt loop
                # can start on head 0 before the others are done
                nc.scalar.activation(
                    wbc[:, h * rows:(h + 1) * rows],
                    sbc[:, h * rows:(h + 1) * rows], AF.Exp, scale=-0.8)

        # ---- Phase 1b: feat matmuls -> fe panels [feat(64) | 1 | 1]; t -> v,q ----
        t16 = big.tile([P, ntiles * 4], F32, tag="t16")
        t3 = t16[:].rearrange("p (n c) -> p n c", c=4)
        t02t = big.tile([P, ntiles * 4], F32, tag="t02t")
        t02 = t02t[:].rearrange("p (n c) -> p n c", c=4)
        vq = big.tile([P, ntiles * 8], F32, tag="vq")
        vq3 = vq[:].rearrange("p (n c) -> p n c", c=8)
        fe = big.tile([P, ntiles * FET_C], F16, tag="fe")
        fe3 = fe[:].rearrange("p (n c) -> p n c", c=FET_C)
        # ones columns (64, 65 of each 66-group): one strided memset
        fe4 = fe[:].rearrange("p (n g c) -> p n g c", g=4, c=GRP)
        nc.gpsimd.memset(fe4[:, :, :, 64:66], 1.0)

        # ---- Fused main loop: feat production interleaved with the masked
        # accumulation so no engine queues a whole phase ahead of another.
        # per chunk of 4 n-tiles: feat matmuls + fe/t3/vq, then two jt-pairs:
        #   u[h] = max(wbc_h * v_j, q_j)  (DVE 2x, some tiles on ACT as
        #   exp(0.8*relu(s+t)+0.2t) — identical values, routes mix freely),
        #   G = u * A_row (one batched DVE 2x op, A broadcast across heads),
        #   then per head a matmul moving=G_h [128,512], stationary=[feat|1]
        #   (65 cols) accumulating psum_h[65, 512] over all jt.
        u_pool = ctx.enter_context(tc.tile_pool(name="u", bufs=3))
        g_pool = ctx.enter_context(tc.tile_pool(name="g", bufs=2))
        r_pool = ctx.enter_context(tc.tile_pool(name="r", bufs=6))
        e_pool = ctx.enter_context(tc.tile_pool(name="epi", bufs=8))
        out_sb_pool = ctx.enter_context(tc.tile_pool(name="osb", bufs=4))
        out_sbs = []
        for ib in range(nib):
            osb = out_sb_pool.tile([P, HEADS * OUT_DIM], F32, tag="outsb")
            out_sbs.append(osb)
        wbc_v = wbc[:].rearrange("p (h i) -> p h i", i=rows)
        CHUNK = 4   # n-tiles per feat/exp chunk
        JB = 4      # j-tiles per batched A-mask op

        def emit_feat_chunk(pf, nt0):
            head = nt0 < 2 * CHUNK   # first chunks: shortest critical path
            for nt in range(nt0, nt0 + CHUNK):
                ps = pf.tile([P, 264], F32, tag="ps")
                for d in range(2):
                    nc.tensor.matmul(
                        ps[:, 0:260],
                        xt_sb[:, d * n_total + nt * P: d * n_total + (nt + 1) * P],
                        w8_sl(d, 0, 260),
                        start=(d == 0), stop=(d == 1))
                nc.scalar.activation(t3[:, nt, :], ps[:, 256:260], AF.Copy)
                if head:
                    # v,q straight from psum; skips the t3 serialization
                    nc.scalar.activation(vq3[:, nt, 0:4], ps[:, 256:260],
                                         AF.Exp)
                    nc.scalar.activation(vq3[:, nt, 4:8], ps[:, 256:260],
                                         AF.Exp, scale=0.2)
                fe_g = fe3[:, nt, :].rearrange("p (g c) -> p g c", c=GRP)
                nc.scalar.activation(
                    fe_g[:, :, 0:64],
                    ps[:, 0:256].rearrange("p (g c) -> p g c", c=64),
                    AF.Copy)
            ch = slice(nt0, nt0 + CHUNK)
            if not head:
                nc.scalar.activation(vq3[:, ch, 0:4], t3[:, ch, :], AF.Exp)
                nc.scalar.activation(vq3[:, ch, 4:8], t3[:, ch, :], AF.Exp,
                                     scale=0.2)
            # 0.2*t, used as the bias of the ACT-route u pass
            nc.scalar.activation(t02[:, ch, :], t3[:, ch, :], AF.Copy,
                                 scale=0.2)

        def emit_u(jt0):
            """u tiles for pair jt0; returns the 4-d u view."""
            u = u_pool.tile([P, JB * HEADS * rows], F16, tag="u")
            u4 = u[:].rearrange("p (j h i) -> p j h i", h=HEADS, i=rows)
            for jt in range(jt0, jt0 + JB):
                for h in range(HEADS):
                    on_act = (h == 3)
                    if on_act:
                        r = r_pool.tile([P, rows], F16, tag="r")
                        nc.scalar.activation(
                            r[:], sbc[:, h * rows:(h + 1) * rows],
                            AF.Relu, scale=-1.0,
                            bias=t3[:, jt, h:h + 1])
                        nc.scalar.activation(
                            u4[:, jt - jt0, h, :], r[:],
                            AF.Exp, scale=0.8,
                            bias=t02[:, jt, h:h + 1])
                    else:
                        nc.vector.tensor_scalar(
                            u4[:, jt - jt0, h, :], wbc_v[:, h, :],
                            vq3[:, jt, h:h + 1], vq3[:, jt, 4 + h:5 + h],
                            AL.mult, AL.max)
            return u4

        def emit_gmm(pacc, jt0, u4):
            """A-mask + accumulation matmuls for pair jt0."""
            g = g_pool.tile([P, JB * HEADS * rows], F16, tag="g")
            g4 = g[:].rearrange("p (j h i) -> p j h i", h=HEADS, i=rows)
            a_b = at_sb[:, jt0 * rows:(jt0 + JB) * rows].rearrange(
                "p (j o i) -> p j o i", j=JB, o=1).to_broadcast(
                (P, JB, HEADS, rows))
            nc.vector.tensor_tensor(g4[:, :, :, :], u4[:, :, :, :],
                                    a_b, AL.mult)
            for jt in range(jt0, jt0 + JB):
                for h in range(HEADS):
                    nc.tensor.matmul(
                        pacc[h][0:65, :],
                        fe3[:, jt, h * GRP: h * GRP + 65],
                        g4[:, jt - jt0, h, :],
                        start=(jt == 0), stop=(jt == njt - 1))

        with tc.tile_pool(name="pacc", bufs=1, space=bass.MemorySpace.PSUM) as pA:
            pacc = []
            for h in range(HEADS):
                pacc_h = pA.tile([P, rows], F32, tag=f"pacc{h}")
                pacc.append(pacc_h)
            # feat chunks prefetched 2 ahead of the jt-pairs that consume
            # them, so the in-order PE queue never makes a chunk's masks wait
            # on the previous chunk's accumulation matmuls.
            LEAD = 2
            nchunks = ntiles // CHUNK
            with tc.tile_pool(name="pfeat", bufs=3,
                              space=bass.MemorySpace.PSUM) as pf:
                for k in range(min(LEAD, nchunks)):
                    emit_feat_chunk(pf, k * CHUNK)
                # software-pipeline: u for pair p+1 is emitted before the
                # A-mask/matmuls of pair p, so the in-order DVE queue always
                # has ready u work while the TT at its head waits on the
                # ACT-route u tiles.
                pend = None   # (jt0, u4) whose gmm is not yet emitted
                for k in range(nchunks):
                    if k + LEAD < nchunks:
                        emit_feat_chunk(pf, (k + LEAD) * CHUNK)
                    for jt0 in range(k * CHUNK, (k + 1) * CHUNK, JB):
                        u4 = emit_u(jt0)
                        if pend is not None:
                            emit_gmm(pacc, pend[0], pend[1])
                        pend = (jt0, u4)
                emit_gmm(pacc, pend[0], pend[1])

            # ---- Epilogue: transpose [65, 512] -> [i, 65]; divide ----
            with tc.tile_pool(name="ptr", bufs=4,
                              space=bass.MemorySpace.PSUM) as pT:
                ths = []
                for h in range(HEADS):
                    th = e_pool.tile([P, rows], F32, tag="th")
                    nc.scalar.activation(th[0:65, :], pacc[h][0:65, :], AF.Copy)
                    ths.append(th)
                # ib-major so each out row-block's DMA can start as soon as
                # its four head slices are divided
                for ib in range(nib):
                    for h in range(HEADS):
                        pt = pT.tile([P, 66], F32, tag="pt")
                        nc.tensor.transpose(
                            pt[:, 0:65],
                            ths[h][0:65, ib * P:(ib + 1) * P],
                            idn[0:65, 0:65])
                        rc = e_pool.tile([P, 1], F32, tag="rc")
                        nc.vector.reciprocal(rc[:], pt[:, 64:65])
                        nc.vector.tensor_scalar(
                            out_sbs[ib][:, h * OUT_DIM:(h + 1) * OUT_DIM],
                            pt[:, 0:64], rc[:], None, AL.mult)
                    nc.sync.dma_start(OUT[ib * P:(ib + 1) * P, :],
                                      out_sbs[ib][:])

    nc.compile()
    return nc


def prep_inputs(X, A, W, a, n_total=N_TOTAL, rows=ROWS, n_cores=N_CORES):
    """Host-side sharding / layout prep.  Returns list of per-core in_maps."""
    f16 = np.float16
    X = np.asarray(X, np.float32)
    A = np.asarray(A)
    W = np.asarray(W, np.float32)
    a = np.asarray(a, np.float32)

    XT = np.ascontiguousarray(X.T).astype(f16)
    Wcat = np.ascontiguousarray(W.transpose(1, 0, 2).reshape(IN_DIM, HEADS * OUT_DIM))
    a_src, a_dst = a[:, :OUT_DIM], a[:, OUT_DIM:]
    w_src = np.einsum('hdo,ho->hd', W, a_src).astype(np.float32)
    w_dst = np.einsum('hdo,ho->hd', W, a_dst).astype(np.float32)
    W8 = np.concatenate([Wcat, w_dst.T], axis=1).astype(np.float32)
    WSRCB = np.repeat(-w_src.T[:, :, None], P, axis=2).reshape(IN_DIM, HEADS * P)

    Af = (A > 0).astype(np.float32)
    in_maps = []
    for c in range(n_cores):
        i0 = c * rows
        at = np.ascontiguousarray(Af[i0:i0 + rows, :].T).astype(f16)
        xtown = X[i0:i0 + rows, :].T
        small = np.concatenate([xtown, W8, WSRCB], axis=1).astype(f16)
        in_maps.append({
            "XT": XT, "SMALL": np.ascontiguousarray(small), "AT": at,
        })
    return in_maps


_CACHED_NC = None


def _get_nc():
    global _CACHED_NC
    if _CACHED_NC is None:
        _CACHED_NC = build_program()
    return _CACHED_NC


def kernel(X, A, W, a, b, _trace=False, _trace_kwargs=None):
    nc = _get_nc()
    in_maps = prep_inputs(X, A, W, a)
    kw = {}
    if _trace:
        kw["trace"] = True
        if _trace_kwargs:
            kw.update(_trace_kwargs)
    res = run_bass_kernel_spmd(nc, in_maps, core_ids=list(range(N_CORES)), **kw)
    out = np.concatenate([r["OUT"] for r in res.results], axis=0)
    out = out + np.asarray(b, np.float32).reshape(1, HEADS * OUT_DIM)
    if _trace:
        return out.astype(np.float32), res
    return out.astype(np.float32)
